# revision 9
# baseline (speedup 1.0000x reference)
"""Trainium2 Bass kernel for nn_AttentionAggregator3d.

Math (per batch b):
    zmf = zm.reshape(CM, N)                     # N = D*W*H = 4096 tokens
    q = Wq @ zmf + bq ; k = Wk @ zmf + bk       # (16, N)
    v = Wv @ zmf + bv                           # (128, N)
    A = softmax_n(q^T k)                        # (N, N), softmax over keys n
    out = v @ A^T ; result = zc + gamma * out

Kernel design (v2 — fp16 dataflow, direct q/k):
  * Sharding: 8 cores = batch (2) x query-block (4, 1024 queries each). Each
    core sees its batch's zm rotated so its query block sits at columns
    0:1024 (softmax/PV sum over all keys, so key order is irrelevant).
  * zm ships as fp16 (1 MB/core instead of 2 MB); q/k are computed on the PE
    as fp16 [17, N] tiles with a 17th contraction row folding the bq bias
    exactly (q row 16 = ones, Wk' col 16 = Wk^T bq; per-query terms and
    bq.bk cancel in softmax).  The PE cost model charges matmuls by output
    free-size only, so the K=17 logits contraction costs the same as the
    old K=128 G-trick while dropping 2 MB of SBUF and most LDWEIGHTS rows.
  * E = exp(logits) is written as bf16 (fp16 would overflow: logits reach
    +31).  PV contracts keys on the PE in pure bf16; softmax denominators
    are accumulated chunk-wise into 4 bf16 accumulators (DVE 2x 16-bit
    mode; 7 chunks go to GpSimd), then folded across partitions by
    ones-matmuls at the tail.
  * gamma is folded into the Wv weights host-side (wvt = gamma*Wv^T) and
    gamma*bv into zc (zca), so the tail is: fold s -> DVE reciprocal ->
    K=1 ones broadcast matmul -> multiply + residual add, pipelined in
    256-column quarters straight into the output DMA.  No Ln table, no
    second ACT table load — the ACT engine runs exactly 33 exps (one
    dummy at t=0 prefetches the Exp table during the DMA wait).
  * Inputs are 3 DMAs: zm16 (4 pieces), packed weights wcat = [Wq^T |
    Wk'^T | gamma*Wv^T] fp16, and zca fp32.  Ones vectors are memset.
"""

import os
import sys
import types

import numpy as np

import concourse.bacc as bacc_mod
import concourse.tile as tile
from concourse import mybir
from concourse.bass_utils import run_bass_kernel_spmd

B, CC, CM, P = 2, 128, 128, 16
N = 16 * 16 * 16          # 4096 tokens
MBLK = N // 4             # 1024 queries per core
NCORES = 8
NCHUNK = N // 128         # 32 key chunks of 128

F32 = mybir.dt.float32
F32R = mybir.dt.float32r
F16 = mybir.dt.float16
BF16 = mybir.dt.bfloat16
AF = mybir.ActivationFunctionType
ALU = mybir.AluOpType

LAST_RESULTS = None  # BassKernelResults of the most recent run (for test.py)


def _ensure_ntff_hook() -> bool:
    """The grading image lacks antenv.axon_hooks; synthesize it from the
    boot module's ctypes NTFF driver so trace=True works under axon."""
    try:
        import antenv.axon_hooks  # noqa: F401

        return True
    except ImportError:
        pass
    try:
        import antenv
        from trn_agent_boot.trn_boot import _ntff_profile_via_ctypes

        hook = _ntff_profile_via_ctypes("/opt/axon/libaxon_pjrt.so")
        mod = types.ModuleType("antenv.axon_hooks")
        mod.get_axon_ntff_profile_hook = lambda: hook
        mod.set_axon_ntff_profile_hook = lambda h: None
        sys.modules["antenv.axon_hooks"] = mod
        antenv.axon_hooks = mod
        return hook is not None
    except Exception:
        return False


def _build():
    nc = bacc_mod.Bacc(
        "TRN2",
        target_bir_lowering=False,
        debug=False,
        num_devices=NCORES,
    )

    zm_d = nc.dram_tensor("zm", (CM, N), F16, kind="ExternalInput").ap()
    wcat_d = nc.dram_tensor("wcat", (CM, 161), F16, kind="ExternalInput").ap()
    zca_d = nc.dram_tensor("zca", (CC, MBLK), F32, kind="ExternalInput").ap()
    onesq_d = nc.dram_tensor("onesq", (1, MBLK), F16, kind="ExternalInput").ap()
    out_d = nc.dram_tensor("out", (CC, MBLK), F32, kind="ExternalOutput").ap()

    LAG = int(os.environ.get("BASS_PV_LAG", "3"))

    with tile.TileContext(nc) as tc:
        with (
            tc.tile_pool(name="consts", bufs=1) as consts,
            tc.tile_pool(name="epool", bufs=6) as epool,
            tc.tile_pool(name="lpool", bufs=2, space="PSUM") as lpool,
            tc.tile_pool(name="opool", bufs=1, space="PSUM") as opool,
            tc.tile_pool(name="spool", bufs=1, space="PSUM") as spool,
        ):
            zm_sb = consts.tile([CM, N], F16, tag="zm")
            wcat_sb = consts.tile([CM, 161], F16, tag="wcat")
            q_sb = consts.tile([17, MBLK], F16, tag="q")
            k_sb = consts.tile([17, N], F16, tag="k")
            vt_sb = consts.tile([128, N], BF16, tag="vt")  # chunk j at cols 128j
            zca_sb = consts.tile([CC, MBLK], F32, tag="zca")
            acc = [
                consts.tile([128, MBLK], BF16, tag=f"acc{i}", name=f"acc{i}")
                for i in range(4)
            ]
            onesc = consts.tile([128, 1], BF16, tag="onesc")
            onesr = consts.tile([1, 128], F32R, tag="onesr")
            onesr_f = consts.tile([1, 128], F32, tag="onesrf")
            rvec = consts.tile([1, MBLK], F32R, tag="rvec")
            rb_sb = consts.tile([CC, MBLK], F32, tag="rb")
            tmp_sb = consts.tile([CC, MBLK], F32, tag="tmp")
            out_sb = consts.tile([CC, MBLK], F32, tag="outsb")
            warm = consts.tile([1, 8], F32, tag="warm")

            # ---- t=0 work: constants via memset, Exp table prefetch, DMAs ----
            nc.vector.memset(onesc[:], 1.0)
            nc.vector.memset(onesr_f[:], 1.0)
            nc.vector.tensor_copy(onesr[:], onesr_f[:])
            nc.vector.memset(warm[:], 0.0)
            # dummy exp: pulls the Exp ACT table while input DMAs stream
            nc.scalar.activation(warm[:], warm[:], AF.Exp)

            nc.scalar.dma_start(wcat_sb[:], wcat_d)
            nc.sync.dma_start(zm_sb[:, 0:1024], zm_d[:, 0:1024])
            nc.scalar.dma_start(zm_sb[:, 1024:2048], zm_d[:, 1024:2048])
            nc.sync.dma_start(zm_sb[:, 2048:3072], zm_d[:, 2048:3072])
            nc.scalar.dma_start(zm_sb[:, 3072:4096], zm_d[:, 3072:4096])
            nc.gpsimd.dma_start(q_sb[16:17, :], onesq_d)
            nc.gpsimd.dma_start(zca_sb[:], zca_d)

            wq = wcat_sb[:, 0:16]
            wk = wcat_sb[:, 16:33]
            wvt = wcat_sb[:, 33:161]

            out_ps = opool.tile([CC, MBLK], F32, tag="out")

            def emit_q():
                st = spool.tile([128, MBLK], F32, tag="S")
                for h in range(2):
                    nc.tensor.matmul(
                        st[0:16, h * 512 : (h + 1) * 512],
                        wq,
                        zm_sb[:, h * 512 : (h + 1) * 512],
                        start=True,
                        stop=True,
                    )
                nc.vector.tensor_copy(q_sb[0:16, :], st[0:16, 0:MBLK])

            def emit_k(i):
                st = spool.tile([128, MBLK], F32, tag="S")
                for h in range(2):
                    nc.tensor.matmul(
                        st[0:17, h * 512 : (h + 1) * 512],
                        wk,
                        zm_sb[:, 1024 * i + h * 512 : 1024 * i + (h + 1) * 512],
                        start=True,
                        stop=True,
                    )
                nc.vector.tensor_copy(
                    k_sb[:, 1024 * i : 1024 * (i + 1)], st[0:17, 0:MBLK]
                )

            def emit_vt(i):
                # vt chunk j = (zm chunk j)^T @ (gamma Wv^T) for j in 8i..8i+7
                st = spool.tile([128, MBLK], F32, tag="S")
                for t in range(8):
                    j = 8 * i + t
                    nc.tensor.matmul(
                        st[:, 128 * t : 128 * (t + 1)],
                        zm_sb[:, 128 * j : 128 * (j + 1)],
                        wvt,
                        start=True,
                        stop=True,
                    )
                nc.vector.tensor_copy(vt_sb[:, 1024 * i : 1024 * (i + 1)], st[:])

            emit_q()
            emit_k(0)
            emit_vt(0)

            e_tiles = {}
            for j in range(NCHUNK + LAG):
                if j < NCHUNK:
                    if j == 2:
                        emit_k(1)
                    elif j == 6:
                        emit_vt(1)
                    elif j == 10:
                        emit_k(2)
                    elif j == 14:
                        emit_vt(2)
                    elif j == 18:
                        emit_k(3)
                    elif j == 22:
                        emit_vt(3)
                    # logits^T chunk j: (keys 128, queries 1024), K=17
                    lps = lpool.tile([128, MBLK], F32, tag="L")
                    for h in range(2):
                        nc.tensor.matmul(
                            lps[:, h * 512 : (h + 1) * 512],
                            k_sb[:, 128 * j : 128 * (j + 1)],
                            q_sb[:, h * 512 : (h + 1) * 512],
                            start=True,
                            stop=True,
                        )
                    ej = epool.tile([128, MBLK], BF16, tag="E")
                    nc.scalar.activation(ej[:], lps[:], AF.Exp)
                    e_tiles[j] = ej
                    # softmax denominator: 4 bf16 accumulators; chunks
                    # j%4==2 (except the last) go to GpSimd, rest to DVE
                    a = j % 4
                    eng = nc.gpsimd if (a == 2 and j <= 26) else nc.vector
                    if j < 4:
                        eng.tensor_copy(acc[a][:], ej[:])
                    else:
                        eng.tensor_add(acc[a][:], acc[a][:], ej[:])
                if j >= LAG:
                    jj = j - LAG
                    ej = e_tiles.pop(jj)
                    for h in range(2):
                        nc.tensor.matmul(
                            out_ps[:, h * 512 : (h + 1) * 512],
                            vt_sb[:, 128 * jj : 128 * (jj + 1)],
                            ej[:, h * 512 : (h + 1) * 512],
                            start=(jj == 0),
                            stop=(jj == NCHUNK - 1),
                        )

            # ---- tail: s fold -> 1/s -> gamma-broadcast -> out, in quarters
            sfold = spool.tile([1, MBLK], F32, tag="S")
            rb = lpool.tile([128, MBLK], F32, tag="L")
            for h in range(2):
                for i in range(4):
                    nc.tensor.matmul(
                        sfold[0:1, h * 512 : (h + 1) * 512],
                        onesc[:],
                        acc[i][:, h * 512 : (h + 1) * 512],
                        start=(i == 0),
                        stop=(i == 3),
                        skip_group_check=True,
                    )
            for qtr in range(4):
                sl = slice(qtr * 256, (qtr + 1) * 256)
                with nc.allow_low_precision(reason="f32r 1/s for broadcast matmul"):
                    nc.vector.reciprocal(rvec[:, sl], sfold[0:1, sl])
                nc.tensor.matmul(
                    rb[:, sl], onesr[:], rvec[:, sl],
                    start=True, stop=True, skip_group_check=True,
                )
                nc.vector.tensor_copy(rb_sb[:, sl], rb[:, sl])
                nc.vector.tensor_tensor(
                    tmp_sb[:, sl], out_ps[:, sl], rb_sb[:, sl], op=ALU.mult
                )
                eng = nc.gpsimd if qtr % 2 == 0 else nc.vector
                eng.tensor_tensor(
                    out_sb[:, sl], tmp_sb[:, sl], zca_sb[:, sl], op=ALU.add
                )
                nc.sync.dma_start(out_d[:, sl], out_sb[:, sl])

    nc.compile()
    return nc


_CACHE = {}


def _get_program():
    if "nc" not in _CACHE:
        _CACHE["nc"] = _build()
    return _CACHE["nc"]


def kernel(zc, zm, Wq, bq, Wk, bk, Wv, bv, gamma):
    global LAST_RESULTS
    zc = np.ascontiguousarray(zc, dtype=np.float32)
    zm = np.ascontiguousarray(zm, dtype=np.float32)
    zmf = zm.reshape(B, CM, N)
    zcf = zc.reshape(B, CC, N)

    Wq = np.asarray(Wq, dtype=np.float32)
    Wk = np.asarray(Wk, dtype=np.float32)
    Wv = np.asarray(Wv, dtype=np.float32)
    bq = np.asarray(bq, dtype=np.float32)
    bv = np.asarray(bv, dtype=np.float32)
    gamma_v = np.float32(np.asarray(gamma).reshape(-1)[0])

    # packed weights: [Wq^T | Wk^T | Wk^T bq | gamma Wv^T] as fp16
    wcat = np.concatenate(
        [Wq.T, Wk.T, (Wk.T @ bq).reshape(CM, 1), gamma_v * Wv.T], axis=1
    ).astype(np.float16)
    wcat = np.ascontiguousarray(wcat)
    adv = (gamma_v * bv).reshape(CC, 1)

    zmf16 = [np.ascontiguousarray(zmf[b].astype(np.float16)) for b in range(B)]

    nc = _get_program()

    in_maps = []
    for c in range(NCORES):
        b, jblk = divmod(c, 4)
        m = {
            "zm": np.ascontiguousarray(
                np.roll(zmf16[b], -MBLK * jblk, axis=1)
            ),
            "wcat": wcat,
            "zca": np.ascontiguousarray(
                zcf[b][:, MBLK * jblk : MBLK * (jblk + 1)] + adv
            ),
            "onesq": np.ones((1, MBLK), dtype=np.float16),
        }
        in_maps.append(m)

    trace = bool(int(os.environ.get("BASS_KERNEL_TRACE", "0")))
    if trace and not _ensure_ntff_hook():
        trace = False
    res = run_bass_kernel_spmd(
        nc,
        in_maps,
        core_ids=list(range(NCORES)),
        trace=trace,
    )
    LAST_RESULTS = res

    out = np.empty((B, CC, N), dtype=np.float32)
    for c in range(NCORES):
        b, jblk = divmod(c, 4)
        out[b][:, MBLK * jblk : MBLK * (jblk + 1)] = res.results[c]["out"]
    return out.reshape(zc.shape)


# revision 12
# speedup vs baseline: 1.0589x; 1.0589x over previous
"""Trainium2 Bass kernel for nn_AttentionAggregator3d.

Math (per batch b):
    zmf = zm.reshape(CM, N)                     # N = D*W*H = 4096 tokens
    q = Wq @ zmf + bq ; k = Wk @ zmf + bk       # (16, N)
    v = Wv @ zmf + bv                           # (128, N)
    A = softmax_n(q^T k)                        # (N, N), softmax over keys n
    out = v @ A^T ; result = zc + gamma * out

Kernel design (v3):
  * Sharding: 8 cores = batch (2) x query-block (4, 1024 queries each). Each
    core sees its batch's zm rotated so its query block sits at columns
    0:1024 (softmax/PV sum over all keys, so key order is irrelevant).
  * zm ships as fp16 (1 MB/core instead of 2 MB); q/k are computed on the PE
    as [17, N] tiles with a 17th contraction row folding the bq bias exactly
    (q row 16 = ones, Wk' col 16 = Wk^T bq; per-query and constant terms
    cancel in softmax), so one program covers all bias cases.  The PE
    charges matmuls by output free-size only, so the K=17 logits contraction
    costs the same as a K=128 one while dropping 2 MB of SBUF and most
    LDWEIGHTS rows.
  * q/k/E stay f32r: measured TRN2 runs the f32r-heavy instruction mix at a
    boosted clock (~0.46 ns/row matmuls) while an all-16-bit mix stays at
    ~1 ns/row.  Only the small projection matmuls consume fp16 (zm,
    weights), and vt is bf16 (both proven fast inside an f32r-heavy mix).
  * Softmax denominators are split three ways by chunk: PE ones-matmuls
    accumulate half-0 of every third chunk into a PSUM bank; GpSimd adds
    accumulate two thirds of half-1; DVE adds take the rest; ones-matmuls
    fold the SBUF accumulators at the tail.
  * gamma is folded into the Wv weights host-side (wvt = gamma*Wv^T) and
    gamma*bv into zc (zca).  Tail per 256-wide quarter: 1/s = exp(-ln s)
    on ACT (same table set as the main exps, loaded once at t=0 by a dummy
    exp), K=1 ones broadcast matmul, multiply + residual add, DMA out.
  * Inputs are 4 DMAs: zm16 (4 pieces), packed weights wcat = [Wq^T | Wk'^T
    | gamma*Wv^T] fp16, zca fp32, and a ones row.  Other constants memset.
"""

import os
import sys
import types

import numpy as np

import concourse.bacc as bacc_mod
import concourse.tile as tile
from concourse import mybir
from concourse.bass_utils import run_bass_kernel_spmd

B, CC, CM, P = 2, 128, 128, 16
N = 16 * 16 * 16          # 4096 tokens
MBLK = N // 4             # 1024 queries per core
NCORES = 8
NCHUNK = N // 128         # 32 key chunks of 128

F32 = mybir.dt.float32
F32R = mybir.dt.float32r
F16 = mybir.dt.float16
BF16 = mybir.dt.bfloat16
AF = mybir.ActivationFunctionType
ALU = mybir.AluOpType

LAST_RESULTS = None  # BassKernelResults of the most recent run (for test.py)


def _ensure_ntff_hook() -> bool:
    """The grading image lacks antenv.axon_hooks; synthesize it from the
    boot module's ctypes NTFF driver so trace=True works under axon."""
    try:
        import antenv.axon_hooks  # noqa: F401

        return True
    except ImportError:
        pass
    try:
        import antenv
        from trn_agent_boot.trn_boot import _ntff_profile_via_ctypes

        hook = _ntff_profile_via_ctypes("/opt/axon/libaxon_pjrt.so")
        mod = types.ModuleType("antenv.axon_hooks")
        mod.get_axon_ntff_profile_hook = lambda: hook
        mod.set_axon_ntff_profile_hook = lambda h: None
        sys.modules["antenv.axon_hooks"] = mod
        antenv.axon_hooks = mod
        return hook is not None
    except Exception:
        return False


# Route Exp and Ln to the one table set that holds both, so the kernel pays a
# single ACT_TABLE_LOAD (prefetched by a dummy exp at t=0).
_orig_gat = bacc_mod.get_activation_tables
_COMBINED_SET = "natural_log_exp_and_others"


def _patched_gat(arch):
    tabs = _orig_gat(arch)
    if _COMBINED_SET in tabs:
        for name, fns in tabs.items():
            if name != _COMBINED_SET:
                fns.discard(AF.Exp)
                fns.discard(AF.Ln)
    return tabs


bacc_mod.get_activation_tables = _patched_gat


def _build():
    nc = bacc_mod.Bacc(
        "TRN2",
        target_bir_lowering=False,
        debug=False,
        num_devices=NCORES,
    )

    zm_d = nc.dram_tensor("zm", (CM, N), F16, kind="ExternalInput").ap()
    wcat_d = nc.dram_tensor("wcat", (CM, 161), F16, kind="ExternalInput").ap()
    zca_d = nc.dram_tensor("zca", (CC, MBLK), F32, kind="ExternalInput").ap()
    onesq_d = nc.dram_tensor("onesq", (1, MBLK), F32R, kind="ExternalInput").ap()
    out_d = nc.dram_tensor("out", (CC, MBLK), F32, kind="ExternalOutput").ap()

    LAG = int(os.environ.get("BASS_PV_LAG", "3"))

    with tile.TileContext(nc) as tc:
        with (
            tc.tile_pool(name="consts", bufs=1) as consts,
            tc.tile_pool(name="epool", bufs=6) as epool,
            tc.tile_pool(name="lpool", bufs=2, space="PSUM") as lpool,
            tc.tile_pool(name="opool", bufs=1, space="PSUM") as opool,
            tc.tile_pool(name="spool", bufs=1, space="PSUM") as spool,
            tc.tile_pool(name="qpool", bufs=1, space="PSUM") as qpool,
        ):
            zm_sb = consts.tile([CM, N], F16, tag="zm")
            wcat_sb = consts.tile([CM, 161], F16, tag="wcat")
            q_sb = consts.tile([17, MBLK], F32R, tag="q")
            k_sb = consts.tile([17, N], F32R, tag="k")
            vt_sb = consts.tile([128, N], F32R, tag="vt")  # chunk j at cols 128j
            zca_sb = consts.tile([CC, MBLK], F32, tag="zca")
            acc0 = consts.tile([128, 512], F32R, tag="acc0")  # half-0, DVE
            acc = consts.tile([128, 512], F32R, tag="acc")    # half-1, DVE
            accg = consts.tile([128, 512], F32R, tag="accg")  # half-1, GpSimd
            onesc = consts.tile([128, 1], F32R, tag="onesc")
            onesc_f = consts.tile([128, 1], F32, tag="onescf")
            onesr = consts.tile([1, 128], F32R, tag="onesr")
            onesr_f = consts.tile([1, 128], F32, tag="onesrf")
            lns = consts.tile([1, MBLK], F32, tag="lns")
            rvec = consts.tile([1, MBLK], F32R, tag="rvec")
            rb_sb = consts.tile([CC, MBLK], F32, tag="rb")
            tmp_sb = consts.tile([CC, MBLK], F32, tag="tmp")
            out_sb = consts.tile([CC, MBLK], F32, tag="outsb")
            warm = consts.tile([1, 8], F32, tag="warm")

            # ---- t=0 work: constants via memset, Exp table prefetch, DMAs ----
            nc.vector.memset(onesc_f[:], 1.0)
            nc.vector.tensor_copy(onesc[:], onesc_f[:])
            nc.vector.memset(onesr_f[:], 1.0)
            nc.vector.tensor_copy(onesr[:], onesr_f[:])
            nc.vector.memset(warm[:], 0.0)
            # dummy exp: pulls the Exp/Ln ACT table while input DMAs stream
            nc.scalar.activation(warm[:], warm[:], AF.Exp)

            nc.scalar.dma_start(wcat_sb[:], wcat_d)
            nc.sync.dma_start(zm_sb[:, 0:1024], zm_d[:, 0:1024])
            nc.scalar.dma_start(zm_sb[:, 1024:2048], zm_d[:, 1024:2048])
            nc.sync.dma_start(zm_sb[:, 2048:3072], zm_d[:, 2048:3072])
            nc.scalar.dma_start(zm_sb[:, 3072:4096], zm_d[:, 3072:4096])
            nc.gpsimd.dma_start(q_sb[16:17, :], onesq_d)
            nc.gpsimd.dma_start(zca_sb[:], zca_d)

            wq = wcat_sb[:, 0:16]
            wk = wcat_sb[:, 16:33]
            wvt = wcat_sb[:, 33:161]

            out_ps = opool.tile([CC, MBLK], F32, tag="out")
            # half-0 denominator sums for j%3==0 chunks, accumulated on PE
            s_ps = qpool.tile([1, 512], F32, tag="s")

            def emit_q(h):
                st = spool.tile([128, 512], F32, tag="S")
                nc.tensor.matmul(
                    st[0:16, :],
                    wq,
                    zm_sb[:, h * 512 : (h + 1) * 512],
                    start=True,
                    stop=True,
                )
                nc.vector.tensor_copy(
                    q_sb[0:16, h * 512 : (h + 1) * 512], st[0:16, :]
                )

            def emit_k(i, h):
                st = spool.tile([128, 512], F32, tag="S")
                c0 = 1024 * i + h * 512
                nc.tensor.matmul(
                    st[0:17, :], wk, zm_sb[:, c0 : c0 + 512], start=True, stop=True
                )
                nc.vector.tensor_copy(k_sb[:, c0 : c0 + 512], st[0:17, :])

            def emit_vt(i):
                # vt chunk j = (zm chunk j)^T @ (gamma Wv^T) for j in 4i..4i+3
                st = spool.tile([128, 512], F32, tag="S")
                for t in range(4):
                    j = 4 * i + t
                    nc.tensor.matmul(
                        st[:, 128 * t : 128 * (t + 1)],
                        zm_sb[:, 128 * j : 128 * (j + 1)],
                        wvt,
                        start=True,
                        stop=True,
                    )
                nc.vector.tensor_copy(vt_sb[:, 512 * i : 512 * (i + 1)], st[:])

            emit_q(0)
            emit_q(1)
            emit_k(0, 0)
            emit_k(0, 1)
            emit_vt(0)

            # stage-buffer emission schedule: before-chunk index -> piece
            emits = {
                1: lambda: emit_k(1, 0),
                2: lambda: emit_vt(1),
                4: lambda: emit_k(1, 1),
                6: lambda: emit_vt(2),
                8: lambda: emit_k(2, 0),
                10: lambda: emit_vt(3),
                12: lambda: emit_k(2, 1),
                14: lambda: emit_vt(4),
                16: lambda: emit_k(3, 0),
                18: lambda: emit_vt(5),
                20: lambda: emit_k(3, 1),
                22: lambda: emit_vt(6),
                25: lambda: emit_vt(7),
            }

            e_tiles = {}
            for j in range(NCHUNK + LAG):
                if j < NCHUNK:
                    if j in emits:
                        emits[j]()
                    # logits^T chunk j: (keys 128, queries 1024), K=17
                    lps = lpool.tile([128, MBLK], F32, tag="L")
                    for h in range(2):
                        nc.tensor.matmul(
                            lps[:, h * 512 : (h + 1) * 512],
                            k_sb[:, 128 * j : 128 * (j + 1)],
                            q_sb[:, h * 512 : (h + 1) * 512],
                            start=True,
                            stop=True,
                        )
                    ej = epool.tile([128, MBLK], F32R, tag="E")
                    nc.scalar.activation(ej[:], lps[:], AF.Exp)
                    e_tiles[j] = ej
                    # denominator half-0: PE for j%3==0, DVE acc0 otherwise
                    if j % 3 == 0:
                        nc.tensor.matmul(
                            s_ps[0:1, :],
                            onesc[:],
                            ej[:, 0:512],
                            start=(j == 0),
                            stop=False,
                            skip_group_check=True,
                        )
                    elif j == 1:
                        nc.vector.tensor_copy(acc0[:], ej[:, 0:512])
                    else:
                        nc.vector.tensor_add(acc0[:], acc0[:], ej[:, 0:512])
                    # denominator half-1: GpSimd for j%3!=0, DVE for j%3==0
                    if j % 3 != 0:
                        if j == 1:
                            nc.gpsimd.tensor_copy(accg[:], ej[:, 512:1024])
                        else:
                            nc.gpsimd.tensor_add(accg[:], accg[:], ej[:, 512:1024])
                    else:
                        if j == 0:
                            nc.vector.tensor_copy(acc[:], ej[:, 512:1024])
                        else:
                            nc.vector.tensor_add(acc[:], acc[:], ej[:, 512:1024])
                if j >= LAG:
                    jj = j - LAG
                    ej = e_tiles.pop(jj)
                    for h in range(2):
                        nc.tensor.matmul(
                            out_ps[:, h * 512 : (h + 1) * 512],
                            vt_sb[:, 128 * jj : 128 * (jj + 1)],
                            ej[:, h * 512 : (h + 1) * 512],
                            start=(jj == 0),
                            stop=(jj == NCHUNK - 1),
                        )

            # ---- tail: s fold -> 1/s -> broadcast -> out, in quarters ----
            # half-0: fold acc0 into the PE-accumulated s_ps bank
            nc.tensor.matmul(
                s_ps[0:1, :], onesc[:], acc0[:],
                start=False, stop=True, skip_group_check=True,
            )
            # half-1: fold acc + accg into a stage-bank tile
            sf = spool.tile([1, 512], F32, tag="S")
            nc.tensor.matmul(
                sf[0:1, :], onesc[:], acc[:],
                start=True, stop=False, skip_group_check=True,
            )
            nc.tensor.matmul(
                sf[0:1, :], onesc[:], accg[:],
                start=False, stop=True, skip_group_check=True,
            )
            rb = lpool.tile([128, MBLK], F32, tag="L")
            for qtr in range(4):
                sl = slice(qtr * 256, (qtr + 1) * 256)
                if qtr < 2:
                    s_src = s_ps[0:1, sl]
                else:
                    s_src = sf[0:1, qtr * 256 - 512 : (qtr + 1) * 256 - 512]
                # 1/s = exp(-ln s), same ACT table set as the main exps
                nc.scalar.activation(lns[:, sl], s_src, AF.Ln)
                nc.scalar.activation(rvec[:, sl], lns[:, sl], AF.Exp, scale=-1.0)
                nc.tensor.matmul(
                    rb[:, sl], onesr[:], rvec[:, sl],
                    start=True, stop=True, skip_group_check=True,
                )
                nc.vector.tensor_copy(rb_sb[:, sl], rb[:, sl])
                nc.vector.tensor_tensor(
                    tmp_sb[:, sl], out_ps[:, sl], rb_sb[:, sl], op=ALU.mult
                )
                eng = nc.gpsimd if qtr % 2 == 0 else nc.vector
                eng.tensor_tensor(
                    out_sb[:, sl], tmp_sb[:, sl], zca_sb[:, sl], op=ALU.add
                )
                nc.sync.dma_start(out_d[:, sl], out_sb[:, sl])

    nc.compile()
    return nc


_CACHE = {}


def _get_program():
    if "nc" not in _CACHE:
        _CACHE["nc"] = _build()
    return _CACHE["nc"]


def kernel(zc, zm, Wq, bq, Wk, bk, Wv, bv, gamma):
    global LAST_RESULTS
    zc = np.ascontiguousarray(zc, dtype=np.float32)
    zm = np.ascontiguousarray(zm, dtype=np.float32)
    zmf = zm.reshape(B, CM, N)
    zcf = zc.reshape(B, CC, N)

    Wq = np.asarray(Wq, dtype=np.float32)
    Wk = np.asarray(Wk, dtype=np.float32)
    Wv = np.asarray(Wv, dtype=np.float32)
    bq = np.asarray(bq, dtype=np.float32)
    bv = np.asarray(bv, dtype=np.float32)
    gamma_v = np.float32(np.asarray(gamma).reshape(-1)[0])

    # packed weights: [Wq^T | Wk^T | Wk^T bq | gamma Wv^T] as fp16
    wcat = np.concatenate(
        [Wq.T, Wk.T, (Wk.T @ bq).reshape(CM, 1), gamma_v * Wv.T], axis=1
    ).astype(np.float16)
    wcat = np.ascontiguousarray(wcat)
    adv = (gamma_v * bv).reshape(CC, 1)

    zmf16 = [np.ascontiguousarray(zmf[b].astype(np.float16)) for b in range(B)]

    nc = _get_program()

    in_maps = []
    for c in range(NCORES):
        b, jblk = divmod(c, 4)
        m = {
            "zm": np.ascontiguousarray(
                np.roll(zmf16[b], -MBLK * jblk, axis=1)
            ),
            "wcat": wcat,
            "zca": np.ascontiguousarray(
                zcf[b][:, MBLK * jblk : MBLK * (jblk + 1)] + adv
            ),
            "onesq": np.ones((1, MBLK), dtype=np.float32),
        }
        in_maps.append(m)

    trace = bool(int(os.environ.get("BASS_KERNEL_TRACE", "0")))
    if trace and not _ensure_ntff_hook():
        trace = False
    res = run_bass_kernel_spmd(
        nc,
        in_maps,
        core_ids=list(range(NCORES)),
        trace=trace,
    )
    LAST_RESULTS = res

    out = np.empty((B, CC, N), dtype=np.float32)
    for c in range(NCORES):
        b, jblk = divmod(c, 4)
        out[b][:, MBLK * jblk : MBLK * (jblk + 1)] = res.results[c]["out"]
    return out.reshape(zc.shape)
